# revision 9
# baseline (speedup 1.0000x reference)
"""BiSPA (bidirectional sparse windowed attention + MLP) Trainium2 kernel.

Full inputs in, full outputs out. Internally shards across 8 NeuronCores:
core c owns output rows Ic = [24c, 24c+24) of the (192, 192, 512) grid.

Key observation: with B == S == 192 and window W == 32,
  - vertical attention for output row i is a complete 192-token sliding-window
    attention over x[i, :, :]                        (needs x rows  Ic)
  - horizontal attention for output row i is a complete 192-token
    sliding-window attention with Q from x[i, :, :] and K/V from x[:, i, :]
    (needs x columns Ic)
so each core needs x[Ic, :, :] and x[:, Ic, :] and NOTHING else -> zero
duplicated projection FLOPs, zero collectives, no halos.

Numerics: all matmul inputs bf16, fp32 PSUM accumulation, fp32 softmax exp
input, bf16 probs/ctx.  Measured end-to-end rel err vs fp32 reference ~0.6%.

Per strip (one row index, 192 tokens, processed identically for both
branches):
  QK^T feature-major (d on partitions)   -> scores S^T = K @ Q^T (keys on
  partitions, q on free), banded in two blocks:
     TA: keys[0:128)   x q[0:128)   (mask: band & ~(k>=96 & q>=96))
     TB: keys[64:192)  x q[96:192)  (mask: band & k>=96)
  exp (ACT, scale=1/8, fp32->bf16) -> band-mask multiply (DVE) -> P
  V token-major with a ones-column per head (V') -> attn@V token-major
  (5 matmuls/head; ones-column accumulates the softmax denominator Z)
  -> normalize by 1/Z via per-partition tensor_scalar -> transpose ctx to
  feature-major (PE transpose) -> fused MLP.

v2: the out-projections are folded into the MLP's first matmul on the host
(hid = relu(W1h' @ h_ctx + W1v' @ v_ctx + b1_eff) with W1h' = W1h @ h_out_w,
W1v' = W1v @ v_out_w) -- exact algebra, kills 14% of PE cycles. The
normalize uses 2 free-dim-broadcast tensor_tensors instead of 4
tensor_scalars per head pair.
"""

import numpy as np
from contextlib import ExitStack

import concourse.bass as bass
import concourse.mybir as mybir
import concourse.tile as tile
from concourse import bacc
from concourse.bass_utils import run_bass_kernel_spmd
from concourse.masks import make_identity
from concourse.tile import add_dep_helper


def _chain(insts):
    """Order matmuls targeting one PSUM bank: a start=True zeroes (marks
    pending-zero) the WHOLE 2KB bank, so each bank must hold exactly one
    accumulation group and the group's matmuls must execute in program order.
    Tile won't order disjoint-region writes by itself."""
    for a, b in zip(insts, insts[1:]):
        add_dep_helper(b.ins, a.ins, sync=False, reason="psum-bank group order")

BF = mybir.dt.bfloat16
F32 = mybir.dt.float32
AF = mybir.ActivationFunctionType
MUL = mybir.AluOpType.mult
NPBF = mybir.dt.np(BF)

E = 512
H = 8
D = 64
W = 32
S = 192
NCORE = 8
RPC = 24          # rows (strips) per core
T = RPC * S       # tokens per core per branch = 4608

# ctx transpose path: "pe" (TensorE transpose) or "dma" (xbar DMA transpose)
CTX_TRANSPOSE = "dma"


def _band_masks():
    """Score mask, bf16 (128, 448): [TA 128 | TB 96 | pad] x 2 heads.

    TA: rows p = key k in [0,128), cols q in [0,128).
        valid = |k-q| <= W and not (k >= 96 and q >= 96)
    TB: rows p -> key k = 64+p in [64,192), cols q' -> q = 96+q' in [96,192).
        valid = |k-q| <= W and k >= 96
    """
    m = np.zeros((128, 224), np.float32)
    k = np.arange(128)[:, None]
    q = np.arange(128)[None, :]
    ta = (np.abs(k - q) <= W) & ~((k >= 96) & (q >= 96))
    kb = 64 + np.arange(128)[:, None]
    qb = 96 + np.arange(96)[None, :]
    tb = (np.abs(kb - qb) <= W) & (kb >= 96)
    m[:, 0:128] = ta
    m[:, 128:224] = tb
    return np.concatenate([m, m], axis=1).astype(NPBF)


def _build_program(bias_flags):
    """Build the SPMD Bass/Tile program (same program on all 8 cores)."""
    has_vqk_b, has_hq_b, has_hk_b, has_b1, has_b2 = bias_flags

    nc = bacc.Bacc("TRN2", target_bir_lowering=False, debug=False,
                   num_devices=NCORE, num_swdge_queues=4)

    xr_t = nc.dram_tensor("xr_t", [E, T], BF, kind="ExternalInput").ap()
    xc_t = nc.dram_tensor("xc_t", [E, T], BF, kind="ExternalInput").ap()
    w_vin = nc.dram_tensor("w_vin", [E, 3 * E], BF, kind="ExternalInput").ap()
    w_hq = nc.dram_tensor("w_hq", [E, E], BF, kind="ExternalInput").ap()
    w_hkv = nc.dram_tensor("w_hkv", [E, 2 * E], BF, kind="ExternalInput").ap()
    w_m1 = nc.dram_tensor("w_m1", [2 * E, E], BF, kind="ExternalInput").ap()
    w_m2 = nc.dram_tensor("w_m2", [E, E], BF, kind="ExternalInput").ap()
    mask_d = nc.dram_tensor("mask", [128, 448], BF, kind="ExternalInput").ap()
    bias_d = nc.dram_tensor("biases", [128, 32], F32, kind="ExternalInput").ap()
    out_t = nc.dram_tensor("out_t", [E, T], F32, kind="ExternalOutput").ap()

    with tile.TileContext(nc) as tc, ExitStack() as ctx:
        pw = ctx.enter_context(tc.tile_pool(name="pw", bufs=1))
        psA = ctx.enter_context(tc.tile_pool(name="psA", bufs=2, space="PSUM"))
        psS = ctx.enter_context(tc.tile_pool(name="psS", bufs=3, space="PSUM"))
        psC = ctx.enter_context(tc.tile_pool(name="psC", bufs=2, space="PSUM"))
        px = ctx.enter_context(tc.tile_pool(name="px", bufs=3))
        pqk = ctx.enter_context(tc.tile_pool(name="pqk", bufs=32))
        pv = ctx.enter_context(tc.tile_pool(name="pv", bufs=8))
        pp = ctx.enter_context(tc.tile_pool(name="pp", bufs=12))
        pctx = ctx.enter_context(tc.tile_pool(name="pctx", bufs=8))
        pzr = ctx.enter_context(tc.tile_pool(name="pzr", bufs=8))
        pct = ctx.enter_context(tc.tile_pool(name="pct", bufs=16))
        phid = ctx.enter_context(tc.tile_pool(name="phid", bufs=8))
        pout = ctx.enter_context(tc.tile_pool(name="pout", bufs=8))

        # ---- persistent constants ----
        def load_const(name, dram_ap, shape, dtype):
            t = pw.tile(shape, dtype, tag=name)
            nc.gpsimd.dma_start(t[:], dram_ap)
            return t

        wv = [load_const(f"wv{k}", w_vin[128 * k:128 * (k + 1), :], [128, 3 * E], BF)
              for k in range(4)]
        whq = [load_const(f"whq{k}", w_hq[128 * k:128 * (k + 1), :], [128, E], BF)
               for k in range(4)]
        whkv = [load_const(f"whkv{k}", w_hkv[128 * k:128 * (k + 1), :], [128, 2 * E], BF)
                for k in range(4)]
        wm1 = [load_const(f"wm1{k}", w_m1[128 * k:128 * (k + 1), :], [128, E], BF)
               for k in range(8)]
        wm2 = [load_const(f"wm2{k}", w_m2[128 * k:128 * (k + 1), :], [128, E], BF)
               for k in range(4)]
        msk = load_const("msk", mask_d[:, :], [128, 448], BF)
        bia = load_const("bia", bias_d[:, :], [128, 32], F32)
        ident = pw.tile([128, 128], BF, tag="ident")
        make_identity(nc, ident)

        # bias column map (within `bia`):
        # 0-7 v_in_b[0:1024] ftiles; 8-11 h_in_b[0:512]; 12-15 h_in_b[512:1024]
        # 16-19 h_out_eff; 20-23 v_out_eff; 24-27 mlp_b1; 28-31 mlp_b2

        def evict384(ps, dst_pool, tag, dtype, func, has_bias, bias_base):
            """Evict a (128, 384) PSUM region (two 192-col ftile halves) to
            SBUF via ScalarE, optionally adding per-ftile per-partition bias."""
            dst = dst_pool.tile([128, 384], dtype, tag=tag)
            if has_bias:
                for half in range(2):
                    nc.scalar.activation(
                        dst[:, 192 * half:192 * half + 192],
                        ps[:, 192 * half:192 * half + 192],
                        func, bias=bia[:, bias_base + half:bias_base + half + 1])
            else:
                nc.scalar.activation(dst[:, 0:384], ps[:, 0:384], func)
            return dst

        import os as _os
        NPAIR = int(_os.environ.get("BISPA_NPAIRS", RPC // 2))
        for g in range(NPAIR):
            g0 = 2 * S * g
            # ---- stage X^T for the strip pair (384 tokens each) ----
            xr2 = []
            xc2 = []
            for k in range(4):
                t = px.tile([128, 2 * S], BF, tag=f"xr{k}")
                nc.gpsimd.dma_start(t[:], xr_t[128 * k:128 * (k + 1), g0:g0 + 2 * S])
                xr2.append(t)
                t = px.tile([128, 2 * S], BF, tag=f"xc{k}")
                nc.gpsimd.dma_start(t[:], xc_t[128 * k:128 * (k + 1), g0:g0 + 2 * S])
                xc2.append(t)

            # ---------- QK projections, feature-major, N=384 ----------
            # ftile j in 0..7: j<4 -> Q features, j>=4 -> K features
            qk = {}
            for br in ("h", "v"):
                qk[br] = []
                for j in range(8):
                    ps = psA.tile([128, 384], F32, tag="proj",
                                  padded_shape=[128, 512])
                    for k in range(4):
                        if br == "v":
                            lhsT = wv[k][:, 128 * j:128 * (j + 1)]
                            rhs = xr2[k][:]
                        elif j < 4:   # h Q
                            lhsT = whq[k][:, 128 * j:128 * (j + 1)]
                            rhs = xr2[k][:]
                        else:         # h K
                            lhsT = whkv[k][:, 128 * (j - 4):128 * (j - 3)]
                            rhs = xc2[k][:]
                        nc.tensor.matmul(ps[:], lhsT=lhsT, rhs=rhs,
                                         start=(k == 0), stop=(k == 3))
                    # bias columns: v ftiles 0-7 -> cols 0-7; h Q 0-3 -> 8-11;
                    # h K 0-3 -> 12-15
                    bcol = j if br == "v" else (8 + j)
                    dst = pqk.tile([128, 384], BF, tag="qk")
                    nc.scalar.activation(dst[:], ps[:], AF.Identity,
                                         bias=bia[:, bcol:bcol + 1])
                    qk[br].append(dst)

            ct = {"h": [], "v": []}   # ctx^T tiles (128, 384), per c-ftile
            for br in ("h", "v"):
                for p in range(4):
                    ct_t = pct.tile([128, 2 * S], BF, tag="ct", name=f"ct_{br}_{g}_{p}")
                    ct[br].append(ct_t)

            for a in range(2):        # strip within the pair
                s0 = S * a
                for br in ("h", "v"):
                    xin = xr2 if br == "v" else xc2
                    vcols = slice(1024, 1536) if br == "v" else slice(512, 1024)
                    vw = wv if br == "v" else whkv
                    # ------ V projection, token-major, with ones column ----
                    vps_a = psA.tile([128, 512], F32, tag="proj")
                    vps_b = psA.tile([128, 512], F32, tag="proj")
                    for k in range(4):
                        nc.tensor.matmul(vps_a[:], lhsT=xin[k][:, s0:s0 + 128],
                                         rhs=vw[k][:, vcols],
                                         start=(k == 0), stop=(k == 3))
                    for k in range(4):
                        nc.tensor.matmul(vps_b[:], lhsT=xin[k][:, s0 + 64:s0 + 192],
                                         rhs=vw[k][:, vcols],
                                         start=(k == 0), stop=(k == 3))
                    va = pv.tile([128, 8, 65], BF, tag="vp")   # keys [0:128)
                    vb = pv.tile([128, 8, 65], BF, tag="vp")   # keys [64:192)
                    nc.vector.tensor_copy(
                        va[:, :, 0:64],
                        vps_a[:].rearrange("p (h c) -> p h c", c=64))
                    nc.vector.tensor_copy(
                        vb[:, :, 0:64],
                        vps_b[:].rearrange("p (h c) -> p h c", c=64))
                    nc.vector.memset(va[:, :, 64:65], 1.0)
                    nc.vector.memset(vb[:, :, 64:65], 1.0)

                    # ------ attention, head-pair-wise ------
                    for p in range(4):
                        QT = qk[br][p][:, s0:s0 + S]
                        KT = qk[br][4 + p][:, s0:s0 + S]

                        # Scores per head in its OWN psum bank: matmuls with
                        # disjoint contraction row-groups (head0 at partitions
                        # 0:64, head1 at 64:128) run CONCURRENTLY on the PE
                        # and hard-fault if they write the same PSUM bank.
                        # Separate banks make the concurrency a ~2x PE win.
                        sps = []
                        for h2 in range(2):
                            d0 = 64 * h2
                            sp = psS.tile([128, 512], F32, tag="sc")
                            nc.tensor.matmul(sp[:, 0:128],
                                             lhsT=KT[d0:d0 + 64, 0:128],
                                             rhs=QT[d0:d0 + 64, 0:128],
                                             start=True, stop=True)
                            nc.tensor.matmul(sp[:, 128:224],
                                             lhsT=KT[d0:d0 + 64, 64:192],
                                             rhs=QT[d0:d0 + 64, 96:192],
                                             start=True, stop=True)
                            sps.append(sp)
                        pb = pp.tile([128, 512], BF, tag="p")
                        for h2 in range(2):
                            nc.scalar.activation(pb[:, 224 * h2:224 * h2 + 224],
                                                 sps[h2][:, 0:224], AF.Exp,
                                                 scale=0.125)
                        pm = pp.tile([128, 512], BF, tag="p")
                        nc.vector.tensor_tensor(pm[:, 0:448], pb[:, 0:448],
                                                msk[:, 0:448], op=MUL)

                        # attn@V, one accumulation "group" per bank, ordered
                        # by _chain. All matmuls use base-0 contraction rows
                        # padded to overlapping ranges (the pad rows are
                        # band-mask zeros in pm), so none of them can run
                        # concurrently and collide on the bank. The built-in
                        # group checker cannot express multi-region banks, so
                        # skip it; correctness comes from the explicit
                        # ordering + per-element pending-zero semantics
                        # (uniform per instruction here).
                        cp = psC.tile([128, 512], F32, tag="cx")
                        mms = []
                        for h2 in range(2):
                            h = 2 * p + h2
                            cb = 130 * h2
                            ta = 224 * h2
                            tb = 224 * h2 + 128
                            # q in [0,128): keys [0:128) from TA
                            mms.append(nc.tensor.matmul(
                                cp[:, cb:cb + 65], lhsT=pm[:, ta:ta + 128],
                                rhs=va[:, h:h + 1, :], start=(h2 == 0),
                                stop=False, skip_group_check=True))
                            # q in [96,128): keys [96:160) = TB rows [32:96),
                            # K padded to rows [0:96) (rows 0:32 masked 0)
                            mms.append(nc.tensor.matmul(
                                cp[96:128, cb:cb + 65],
                                lhsT=pm[0:96, tb:tb + 32],
                                rhs=vb[0:96, h:h + 1, :],
                                start=False, stop=False, tile_position=(0, 96),
                                skip_group_check=True))
                            # q in [128,192): keys [96:192) = TB rows
                            # [32:128), K padded to rows [0:128)
                            mms.append(nc.tensor.matmul(
                                cp[0:64, cb + 65:cb + 130],
                                lhsT=pm[0:128, tb + 32:tb + 96],
                                rhs=vb[0:128, h:h + 1, :],
                                start=False, stop=(h2 == 1),
                                skip_group_check=True))
                        _chain(mms)

                        # normalize by 1/Z (Z = ones-column accumulation, col
                        # 64 of each 65-block) and pack for transposition:
                        # ctxn = [h0q1 | h1q1 | h0q2 | h1q2], 64 cols each
                        # zr: 0 = h0 q1, 1 = h1 q1 (128 partitions);
                        #     2 = h0 q2, 3 = h1 q2 (partitions [0:64))
                        # single tensor_tensor per q-range: in0 = both heads'
                        # ctx (stride-130 pair), in1 = zr broadcast over d.
                        zr = pzr.tile([128, 4, 1], F32, tag="zr")
                        cp2 = cp[:, 0:260].rearrange("p (x c) -> p x c", c=130)
                        cp2b = cp[0:64, 65:325].rearrange("p (x c) -> p x c",
                                                          c=130)
                        ctxn = pctx.tile([128, 256], BF, tag="ctxn")
                        reads = [
                            nc.vector.reciprocal(zr[:, 0:2, :],
                                                 cp2[:, :, 64:65]),
                            nc.vector.reciprocal(zr[0:64, 2:4, :],
                                                 cp2[0:64, :, 129:130]),
                            nc.vector.tensor_tensor(
                                ctxn[:, 0:128].rearrange("p (a b) -> p a b",
                                                         b=64),
                                cp2[:, :, 0:64],
                                zr[:, 0:2, :].broadcast_to([128, 2, 64]),
                                op=MUL),
                            nc.vector.tensor_tensor(
                                ctxn[0:64, 128:256].rearrange(
                                    "p (a b) -> p a b", b=64),
                                cp2b[:, :, 0:64],
                                zr[0:64, 2:4, :].broadcast_to([64, 2, 64]),
                                op=MUL),
                        ]
                        # cp reads must wait for the accumulation group to
                        # close (same-bank PE-write + DVE-read is a HW fault)
                        for r in reads:
                            add_dep_helper(r.ins, mms[-1].ins, sync=True,
                                           reason="psum read after group close")

                        ct_p = ct[br][p]
                        if CTX_TRANSPOSE == "pe":
                            ctp = psC.tile([128, S], BF, tag="cxT", bufs=1)
                            nc.tensor.transpose(ctp[:, 0:128], ctxn[:, 0:128],
                                                ident[:])
                            nc.tensor.transpose(ctp[:, 128:192],
                                                ctxn[0:64, 128:256],
                                                ident[0:64, 0:64])
                            nc.scalar.activation(ct_p[:, s0:s0 + S], ctp[:],
                                                 AF.Copy)
                        else:
                            nc.sync.dma_start_transpose(ct_p[:, s0:s0 + 128],
                                                        ctxn[:, 0:128])
                            nc.sync.dma_start_transpose(
                                ct_p[:, s0 + 128:s0 + 192],
                                ctxn[0:64, 128:256])

            # ---------- fused out-proj + MLP1, N=384 ----------
            # hid = relu(W1h' @ h_ctx + W1v' @ v_ctx + b1_eff), where
            # W1h' = W1[:, 0:E] @ h_out_w and W1v' = W1[:, E:2E] @ v_out_w
            # are folded on the host. wm1 rows 0:512 act on h ctx,
            # rows 512:1024 on v ctx.
            hid = []
            for j in range(4):
                ps = psA.tile([128, 384], F32, tag="proj",
                              padded_shape=[128, 512])
                for k in range(8):
                    rhs = ct["h"][k] if k < 4 else ct["v"][k - 4]
                    nc.tensor.matmul(ps[:],
                                     lhsT=wm1[k][:, 128 * j:128 * (j + 1)],
                                     rhs=rhs[:],
                                     start=(k == 0), stop=(k == 7))
                dst = phid.tile([128, 384], BF, tag="hid")
                nc.scalar.activation(dst[:], ps[:], AF.Relu,
                                     bias=bia[:, 24 + j:24 + j + 1])
                hid.append(dst)
            for j in range(4):
                ps = psA.tile([128, 384], F32, tag="proj",
                              padded_shape=[128, 512])
                for k in range(4):
                    nc.tensor.matmul(ps[:],
                                     lhsT=wm2[k][:, 128 * j:128 * (j + 1)],
                                     rhs=hid[k][:],
                                     start=(k == 0), stop=(k == 3))
                osb = pout.tile([128, 384], F32, tag="o")
                nc.scalar.activation(osb[:], ps[:], AF.Identity,
                                     bias=bia[:, 28 + j:28 + j + 1])
                nc.sync.dma_start(out_t[128 * j:128 * (j + 1), g0:g0 + 2 * S],
                                  osb[:])
    nc.finalize()
    return nc


_CACHE = {}


def _get_program(bias_flags):
    key = tuple(bias_flags)
    if key not in _CACHE:
        _CACHE[key] = _build_program(key)
    return _CACHE[key]


def _col(b):
    """bias vector (128*n,) -> (128, n) column-pack, fortran-ish layout."""
    return np.ascontiguousarray(b.reshape(-1, 128).T.astype(np.float32))


def kernel(hidden_states, h_in_w, h_in_b, h_out_w, h_out_b,
           v_in_w, v_in_b, v_out_w, v_out_b,
           mlp_w1, mlp_b1, mlp_w2, mlp_b2):
    x = np.asarray(hidden_states, dtype=np.float32)
    h_in_w = np.asarray(h_in_w, np.float32)
    h_in_b = np.asarray(h_in_b, np.float32)
    h_out_w = np.asarray(h_out_w, np.float32)
    h_out_b = np.asarray(h_out_b, np.float32)
    v_in_w = np.asarray(v_in_w, np.float32)
    v_in_b = np.asarray(v_in_b, np.float32)
    v_out_w = np.asarray(v_out_w, np.float32)
    v_out_b = np.asarray(v_out_b, np.float32)
    mlp_w1 = np.asarray(mlp_w1, np.float32)
    mlp_b1 = np.asarray(mlp_b1, np.float32)
    mlp_w2 = np.asarray(mlp_w2, np.float32)
    mlp_b2 = np.asarray(mlp_b2, np.float32)

    # V biases act as a constant shift of ctx (softmax weights sum to 1),
    # so fold them through the out-projections.
    h_out_eff = h_out_b + h_out_w @ h_in_b[2 * E:3 * E]
    v_out_eff = v_out_b + v_out_w @ v_in_b[2 * E:3 * E]

    # Fold the out-projections into the MLP's first matmul (exact algebra):
    # hid = relu(W1h @ (h_ctx @ Who.T + hob) + W1v @ (v_ctx @ Wvo.T + vob)
    #            + b1)
    #     = relu(h_ctx @ (W1h @ Who).T + v_ctx @ (W1v @ Wvo).T + b1_eff)
    w1h = mlp_w1[:, 0:E]
    w1v = mlp_w1[:, E:2 * E]
    w_m1_eff = np.concatenate([w1h @ h_out_w, w1v @ v_out_w], axis=1)
    b1_eff = mlp_b1 + w1h @ h_out_eff + w1v @ v_out_eff

    bias_flags = (
        bool(np.any(v_in_b[0:2 * E])), bool(np.any(h_in_b[0:E])),
        bool(np.any(h_in_b[E:2 * E])), bool(np.any(b1_eff)),
        bool(np.any(mlp_b2)),
    )
    nc = _get_program(bias_flags)

    biases = np.zeros((128, 32), np.float32)
    biases[:, 0:8] = _col(v_in_b[0:2 * E])
    biases[:, 8:16] = _col(h_in_b[0:2 * E])
    biases[:, 24:28] = _col(b1_eff)
    biases[:, 28:32] = _col(mlp_b2)

    shared = {
        "w_vin": np.ascontiguousarray(v_in_w.T).astype(NPBF),
        "w_hq": np.ascontiguousarray(h_in_w[0:E].T).astype(NPBF),
        "w_hkv": np.ascontiguousarray(h_in_w[E:3 * E].T).astype(NPBF),
        "w_m1": np.ascontiguousarray(w_m1_eff.T).astype(NPBF),
        "w_m2": np.ascontiguousarray(mlp_w2.T).astype(NPBF),
        "mask": _band_masks(),
        "biases": biases,
    }

    in_maps = []
    for c in range(NCORE):
        rows = x[RPC * c:RPC * (c + 1)]                      # (24, 192, 512)
        cols = x[:, RPC * c:RPC * (c + 1)].transpose(1, 0, 2)  # (24, 192, 512)
        m = dict(shared)
        m["xr_t"] = np.ascontiguousarray(rows.reshape(T, E).T).astype(NPBF)
        m["xc_t"] = np.ascontiguousarray(cols.reshape(T, E).T).astype(NPBF)
        in_maps.append(m)

    global _LAST_IN_MAPS
    _LAST_IN_MAPS = in_maps
    res = run_bass_kernel_spmd(nc, in_maps, core_ids=list(range(NCORE)))

    out = np.empty((S, S, E), np.float32)
    for c in range(NCORE):
        out[RPC * c:RPC * (c + 1)] = res.results[c]["out_t"].T.reshape(RPC, S, E)
    return out



# revision 15
# speedup vs baseline: 1.7911x; 1.7911x over previous
"""BiSPA (bidirectional sparse windowed attention + MLP) Trainium2 kernel.

Full inputs in, full outputs out. Internally shards across 8 NeuronCores:
core c owns output rows Ic = [24c, 24c+24) of the (192, 192, 512) grid.

Key observation: with B == S == 192 and window W == 32,
  - vertical attention for output row i is a complete 192-token sliding-window
    attention over x[i, :, :]                        (needs x rows  Ic)
  - horizontal attention for output row i is a complete 192-token
    sliding-window attention with Q from x[i, :, :] and K/V from x[:, i, :]
    (needs x columns Ic)
so each core needs x[Ic, :, :] and x[:, Ic, :] and NOTHING else -> zero
duplicated projection FLOPs, zero collectives, no halos.

Numerics: all matmul inputs bf16, fp32 PSUM accumulation, fp32 softmax exp
input, bf16 probs/ctx.  Measured end-to-end rel err vs fp32 reference ~0.6%.

Per strip (one row index, 192 tokens, processed identically for both
branches):
  QK^T feature-major (d on partitions)   -> scores S^T = K @ Q^T (keys on
  partitions, q on free), banded in two blocks:
     TA: keys[0:128)   x q[0:128)   (mask: band & ~(k>=96 & q>=96))
     TB: keys[64:192)  x q[96:192)  (mask: band & k>=96)
  exp (ACT, scale=1/8, fp32->bf16) -> band-mask multiply (DVE) -> P
  V token-major with a ones-column per head (V') -> attn@V token-major
  (5 matmuls/head; ones-column accumulates the softmax denominator Z)
  -> normalize by 1/Z via per-partition tensor_scalar -> transpose ctx to
  feature-major (PE transpose) -> fused MLP.

v2: the out-projections are folded into the MLP's first matmul on the host
(hid = relu(W1h' @ h_ctx + W1v' @ v_ctx + b1_eff) with W1h' = W1h @ h_out_w,
W1v' = W1v @ v_out_w) -- exact algebra, kills 14% of PE cycles. The
normalize uses 2 free-dim-broadcast tensor_tensors instead of 4
tensor_scalars per head pair.
"""

import numpy as np
from contextlib import ExitStack

import concourse.bass as bass
import concourse.mybir as mybir
import concourse.tile as tile
from concourse import bacc
from concourse.bass_utils import run_bass_kernel_spmd
from concourse.masks import make_identity
from concourse.tile import add_dep_helper


def _chain(insts):
    """Order matmuls targeting one PSUM bank: a start=True zeroes (marks
    pending-zero) the WHOLE 2KB bank, so each bank must hold exactly one
    accumulation group and the group's matmuls must execute in program order.
    Tile won't order disjoint-region writes by itself."""
    for a, b in zip(insts, insts[1:]):
        add_dep_helper(b.ins, a.ins, sync=False, reason="psum-bank group order")

BF = mybir.dt.bfloat16
F32 = mybir.dt.float32
AF = mybir.ActivationFunctionType
MUL = mybir.AluOpType.mult
NPBF = mybir.dt.np(BF)

E = 512
H = 8
D = 64
W = 32
S = 192
NCORE = 8
RPC = 24          # rows (strips) per core
T = RPC * S       # tokens per core per branch = 4608

# ctx transpose path: "pe" (TensorE transpose) or "dma" (xbar DMA transpose)
CTX_TRANSPOSE = "pe"


def _band_masks():
    """Score mask, bf16 (128, 384): [TA 96 | TB 96] x 2 heads.

    Symmetric two-rectangle band cover (exact, no overlap):
    TA: rows p = key k in [0,128), cols q in [0,96).   valid = |k-q| <= W
    TB: rows p -> key k = 64+p in [64,192), cols q' -> q = 96+q' in
        [96,192).                                      valid = |k-q| <= W
    """
    m = np.zeros((128, 192), np.float32)
    k = np.arange(128)[:, None]
    q = np.arange(96)[None, :]
    m[:, 0:96] = (np.abs(k - q) <= W)
    kb = 64 + np.arange(128)[:, None]
    qb = 96 + np.arange(96)[None, :]
    m[:, 96:192] = (np.abs(kb - qb) <= W)
    return np.concatenate([m, m], axis=1).astype(NPBF)


def _build_program(bias_flags):
    """Build the SPMD Bass/Tile program (same program on all 8 cores)."""
    has_vqk_b, has_hq_b, has_hk_b, has_b1, has_b2 = bias_flags

    nc = bacc.Bacc("TRN2", target_bir_lowering=False, debug=False,
                   num_devices=NCORE, num_swdge_queues=4)

    xr_t = nc.dram_tensor("xr_t", [E, T], BF, kind="ExternalInput").ap()
    xc_t = nc.dram_tensor("xc_t", [E, T], BF, kind="ExternalInput").ap()
    w_vin = nc.dram_tensor("w_vin", [E, 3 * E], BF, kind="ExternalInput").ap()
    w_hq = nc.dram_tensor("w_hq", [E, E], BF, kind="ExternalInput").ap()
    w_hkv = nc.dram_tensor("w_hkv", [E, 2 * E], BF, kind="ExternalInput").ap()
    w_m1 = nc.dram_tensor("w_m1", [2 * E, E], BF, kind="ExternalInput").ap()
    w_m2 = nc.dram_tensor("w_m2", [E, E], BF, kind="ExternalInput").ap()
    mask_d = nc.dram_tensor("mask", [128, 384], BF, kind="ExternalInput").ap()
    bias_d = nc.dram_tensor("biases", [128, 32], F32, kind="ExternalInput").ap()
    out_t = nc.dram_tensor("out_t", [E, T], F32, kind="ExternalOutput").ap()

    with tile.TileContext(nc) as tc, ExitStack() as ctx:
        pw = ctx.enter_context(tc.tile_pool(name="pw", bufs=1))
        psA = ctx.enter_context(tc.tile_pool(name="psA", bufs=2, space="PSUM"))
        psS = ctx.enter_context(tc.tile_pool(name="psS", bufs=3, space="PSUM"))
        psC = ctx.enter_context(tc.tile_pool(name="psC", bufs=2, space="PSUM"))
        px = ctx.enter_context(tc.tile_pool(name="px", bufs=3))
        pqk = ctx.enter_context(tc.tile_pool(name="pqk", bufs=32))
        pv = ctx.enter_context(tc.tile_pool(name="pv", bufs=8))
        pp = ctx.enter_context(tc.tile_pool(name="pp", bufs=12))
        pctx = ctx.enter_context(tc.tile_pool(name="pctx", bufs=8))
        pzr = ctx.enter_context(tc.tile_pool(name="pzr", bufs=8))
        pct = ctx.enter_context(tc.tile_pool(name="pct", bufs=16))
        phid = ctx.enter_context(tc.tile_pool(name="phid", bufs=8))
        pout = ctx.enter_context(tc.tile_pool(name="pout", bufs=8))

        # ---- persistent constants ----
        def load_const(name, dram_ap, shape, dtype):
            t = pw.tile(shape, dtype, tag=name)
            nc.gpsimd.dma_start(t[:], dram_ap)
            return t

        wv = [load_const(f"wv{k}", w_vin[128 * k:128 * (k + 1), :], [128, 3 * E], BF)
              for k in range(4)]
        whq = [load_const(f"whq{k}", w_hq[128 * k:128 * (k + 1), :], [128, E], BF)
               for k in range(4)]
        whkv = [load_const(f"whkv{k}", w_hkv[128 * k:128 * (k + 1), :], [128, 2 * E], BF)
                for k in range(4)]
        wm1 = [load_const(f"wm1{k}", w_m1[128 * k:128 * (k + 1), :], [128, E], BF)
               for k in range(8)]
        wm2 = [load_const(f"wm2{k}", w_m2[128 * k:128 * (k + 1), :], [128, E], BF)
               for k in range(4)]
        msk = load_const("msk", mask_d[:, :], [128, 384], BF)
        bia = load_const("bia", bias_d[:, :], [128, 32], F32)
        ident = pw.tile([128, 128], BF, tag="ident")
        make_identity(nc, ident)

        # bias column map (within `bia`):
        # 0-7 v_in_b[0:1024] ftiles; 8-11 h_in_b[0:512]; 12-15 h_in_b[512:1024]
        # 16-19 h_out_eff; 20-23 v_out_eff; 24-27 mlp_b1; 28-31 mlp_b2

        def evict384(ps, dst_pool, tag, dtype, func, has_bias, bias_base):
            """Evict a (128, 384) PSUM region (two 192-col ftile halves) to
            SBUF via ScalarE, optionally adding per-ftile per-partition bias."""
            dst = dst_pool.tile([128, 384], dtype, tag=tag)
            if has_bias:
                for half in range(2):
                    nc.scalar.activation(
                        dst[:, 192 * half:192 * half + 192],
                        ps[:, 192 * half:192 * half + 192],
                        func, bias=bia[:, bias_base + half:bias_base + half + 1])
            else:
                nc.scalar.activation(dst[:, 0:384], ps[:, 0:384], func)
            return dst

        import os as _os
        NPAIR = int(_os.environ.get("BISPA_NPAIRS", RPC // 2))
        for g in range(NPAIR):
            g0 = 2 * S * g
            # ---- stage X^T for the strip pair (384 tokens each) ----
            xr2 = []
            xc2 = []
            for k in range(4):
                t = px.tile([128, 2 * S], BF, tag=f"xr{k}")
                nc.gpsimd.dma_start(t[:], xr_t[128 * k:128 * (k + 1), g0:g0 + 2 * S])
                xr2.append(t)
                t = px.tile([128, 2 * S], BF, tag=f"xc{k}")
                nc.gpsimd.dma_start(t[:], xc_t[128 * k:128 * (k + 1), g0:g0 + 2 * S])
                xc2.append(t)

            # ---------- QK projections, feature-major, N=384 ----------
            # ftile j in 0..7: j<4 -> Q features, j>=4 -> K features
            qk = {}
            for br in ("h", "v"):
                qk[br] = []
                for j in range(8):
                    ps = psA.tile([128, 384], F32, tag="proj",
                                  padded_shape=[128, 512])
                    for k in range(4):
                        if br == "v":
                            lhsT = wv[k][:, 128 * j:128 * (j + 1)]
                            rhs = xr2[k][:]
                        elif j < 4:   # h Q
                            lhsT = whq[k][:, 128 * j:128 * (j + 1)]
                            rhs = xr2[k][:]
                        else:         # h K
                            lhsT = whkv[k][:, 128 * (j - 4):128 * (j - 3)]
                            rhs = xc2[k][:]
                        nc.tensor.matmul(ps[:], lhsT=lhsT, rhs=rhs,
                                         start=(k == 0), stop=(k == 3))
                    # bias columns: v ftiles 0-7 -> cols 0-7; h Q 0-3 -> 8-11;
                    # h K 0-3 -> 12-15
                    bcol = j if br == "v" else (8 + j)
                    dst = pqk.tile([128, 384], BF, tag="qk")
                    nc.scalar.activation(dst[:], ps[:], AF.Identity,
                                         bias=bia[:, bcol:bcol + 1])
                    qk[br].append(dst)

            ct = {"h": [], "v": []}   # ctx^T tiles (128, 384), per c-ftile
            for br in ("h", "v"):
                for p in range(4):
                    ct_t = pct.tile([128, 2 * S], BF, tag="ct", name=f"ct_{br}_{g}_{p}")
                    ct[br].append(ct_t)

            for a in range(2):        # strip within the pair
                s0 = S * a
                for br in ("h", "v"):
                    xin = xr2 if br == "v" else xc2
                    vcols = slice(1024, 1536) if br == "v" else slice(512, 1024)
                    vw = wv if br == "v" else whkv
                    # ------ V projection, token-major, with ones column ----
                    vps_a = psA.tile([128, 512], F32, tag="proj")
                    vps_b = psA.tile([128, 512], F32, tag="proj")
                    for k in range(4):
                        nc.tensor.matmul(vps_a[:], lhsT=xin[k][:, s0:s0 + 128],
                                         rhs=vw[k][:, vcols],
                                         start=(k == 0), stop=(k == 3))
                    for k in range(4):
                        nc.tensor.matmul(vps_b[:], lhsT=xin[k][:, s0 + 64:s0 + 192],
                                         rhs=vw[k][:, vcols],
                                         start=(k == 0), stop=(k == 3))
                    va = pv.tile([128, 8, 65], BF, tag="vp")   # keys [0:128)
                    vb = pv.tile([128, 8, 65], BF, tag="vp")   # keys [64:192)
                    nc.vector.tensor_copy(
                        va[:, :, 0:64],
                        vps_a[:].rearrange("p (h c) -> p h c", c=64))
                    nc.vector.tensor_copy(
                        vb[:, :, 0:64],
                        vps_b[:].rearrange("p (h c) -> p h c", c=64))
                    nc.vector.memset(va[:, :, 64:65], 1.0)
                    nc.vector.memset(vb[:, :, 64:65], 1.0)

                    # ------ attention, head-pair-wise ------
                    for p in range(4):
                        QT = qk[br][p][:, s0:s0 + S]
                        KT = qk[br][4 + p][:, s0:s0 + S]

                        # Scores per head in its OWN psum bank: matmuls with
                        # disjoint contraction row-groups (head0 at partitions
                        # 0:64, head1 at 64:128) run CONCURRENTLY on the PE
                        # and hard-fault if they write the same PSUM bank.
                        # Separate banks make the concurrency a ~2x PE win.
                        # Symmetric band cover: TA = keys[0:128) x q[0:96),
                        # TB = keys[64:192) x q[96:192), masks pure band.
                        sps = []
                        for h2 in range(2):
                            d0 = 64 * h2
                            sp = psS.tile([128, 512], F32, tag="sc")
                            nc.tensor.matmul(sp[:, 0:96],
                                             lhsT=KT[d0:d0 + 64, 0:128],
                                             rhs=QT[d0:d0 + 64, 0:96],
                                             start=True, stop=True)
                            nc.tensor.matmul(sp[:, 96:192],
                                             lhsT=KT[d0:d0 + 64, 64:192],
                                             rhs=QT[d0:d0 + 64, 96:192],
                                             start=True, stop=True)
                            sps.append(sp)
                        pb = pp.tile([128, 512], BF, tag="p")
                        for h2 in range(2):
                            nc.scalar.activation(pb[:, 192 * h2:192 * h2 + 192],
                                                 sps[h2][:, 0:192], AF.Exp,
                                                 scale=0.125)
                        pm = pp.tile([128, 512], BF, tag="p")
                        nc.vector.tensor_tensor(pm[:, 0:384], pb[:, 0:384],
                                                msk[:, 0:384], op=MUL)

                        # attn@V: 2 matmuls per head. q[0:96) from keys
                        # [0:128) (va), q[96:192) from keys [64:192) (vb);
                        # all write partitions [0:96) of one bank in 65-col
                        # regions [h0q1 | h1q1 | h0q2 | h1q2] (col 64 of
                        # each = Z from the ones column). One accumulation
                        # "group" per bank, ordered by _chain; the group
                        # checker cannot express multi-region banks, so
                        # skip it.
                        cp = psC.tile([128, 512], F32, tag="cx")
                        mms = []
                        for h2 in range(2):
                            h = 2 * p + h2
                            ta = 192 * h2
                            tb = 192 * h2 + 96
                            mms.append(nc.tensor.matmul(
                                cp[0:96, 65 * h2:65 * h2 + 65],
                                lhsT=pm[:, ta:ta + 96],
                                rhs=va[:, h:h + 1, :], start=(h2 == 0),
                                stop=False, skip_group_check=True))
                            mms.append(nc.tensor.matmul(
                                cp[0:96, 130 + 65 * h2:195 + 65 * h2],
                                lhsT=pm[:, tb:tb + 96],
                                rhs=vb[:, h:h + 1, :],
                                start=False, stop=(h2 == 1),
                                skip_group_check=True))
                        _chain(mms)

                        # normalize by 1/Z, one reciprocal + one broadcast
                        # multiply: ctxn = [h0q1 | h1q1 | h0q2 | h1q2] on
                        # partitions [0:96), transpose-ready.
                        zr = pzr.tile([128, 4, 1], F32, tag="zr")
                        cp4 = cp[0:96, 0:260].rearrange("p (x c) -> p x c",
                                                        c=65)
                        ctxn = pctx.tile([128, 256], BF, tag="ctxn")
                        reads = [
                            nc.vector.reciprocal(zr[0:96, :, :],
                                                 cp4[:, :, 64:65]),
                            nc.vector.tensor_tensor(
                                ctxn[0:96, 0:256].rearrange(
                                    "p (a b) -> p a b", b=64),
                                cp4[:, :, 0:64],
                                zr[0:96, :, :].broadcast_to([96, 4, 64]),
                                op=MUL),
                        ]
                        # cp reads must wait for the accumulation group to
                        # close (same-bank PE-write + DVE-read is a HW fault)
                        for r in reads:
                            add_dep_helper(r.ins, mms[-1].ins, sync=True,
                                           reason="psum read after group close")

                        # transpose q1/q2 blocks into ct (feature-major)
                        ct_p = ct[br][p]
                        ctp = psC.tile([128, S], BF, tag="cxT", bufs=1)
                        nc.tensor.transpose(ctp[:, 0:96], ctxn[0:96, 0:128],
                                            ident[0:96, 0:96])
                        nc.tensor.transpose(ctp[:, 96:192],
                                            ctxn[0:96, 128:256],
                                            ident[0:96, 0:96])
                        nc.scalar.activation(ct_p[:, s0:s0 + S], ctp[:],
                                             AF.Copy)

            # ---------- fused out-proj + MLP1, N=384 ----------
            # hid = relu(W1h' @ h_ctx + W1v' @ v_ctx + b1_eff), where
            # W1h' = W1[:, 0:E] @ h_out_w and W1v' = W1[:, E:2E] @ v_out_w
            # are folded on the host. wm1 rows 0:512 act on h ctx,
            # rows 512:1024 on v ctx.
            hid = []
            for j in range(4):
                ps = psA.tile([128, 384], F32, tag="proj",
                              padded_shape=[128, 512])
                for k in range(8):
                    rhs = ct["h"][k] if k < 4 else ct["v"][k - 4]
                    nc.tensor.matmul(ps[:],
                                     lhsT=wm1[k][:, 128 * j:128 * (j + 1)],
                                     rhs=rhs[:],
                                     start=(k == 0), stop=(k == 7))
                dst = phid.tile([128, 384], BF, tag="hid")
                nc.scalar.activation(dst[:], ps[:], AF.Relu,
                                     bias=bia[:, 24 + j:24 + j + 1])
                hid.append(dst)
            for j in range(4):
                ps = psA.tile([128, 384], F32, tag="proj",
                              padded_shape=[128, 512])
                for k in range(4):
                    nc.tensor.matmul(ps[:],
                                     lhsT=wm2[k][:, 128 * j:128 * (j + 1)],
                                     rhs=hid[k][:],
                                     start=(k == 0), stop=(k == 3))
                osb = pout.tile([128, 384], F32, tag="o")
                nc.scalar.activation(osb[:], ps[:], AF.Identity,
                                     bias=bia[:, 28 + j:28 + j + 1])
                nc.sync.dma_start(out_t[128 * j:128 * (j + 1), g0:g0 + 2 * S],
                                  osb[:])
    nc.finalize()
    return nc


_CACHE = {}


def _get_program(bias_flags):
    key = tuple(bias_flags)
    if key not in _CACHE:
        _CACHE[key] = _build_program(key)
    return _CACHE[key]


def _col(b):
    """bias vector (128*n,) -> (128, n) column-pack, fortran-ish layout."""
    return np.ascontiguousarray(b.reshape(-1, 128).T.astype(np.float32))


def kernel(hidden_states, h_in_w, h_in_b, h_out_w, h_out_b,
           v_in_w, v_in_b, v_out_w, v_out_b,
           mlp_w1, mlp_b1, mlp_w2, mlp_b2):
    x = np.asarray(hidden_states, dtype=np.float32)
    h_in_w = np.asarray(h_in_w, np.float32)
    h_in_b = np.asarray(h_in_b, np.float32)
    h_out_w = np.asarray(h_out_w, np.float32)
    h_out_b = np.asarray(h_out_b, np.float32)
    v_in_w = np.asarray(v_in_w, np.float32)
    v_in_b = np.asarray(v_in_b, np.float32)
    v_out_w = np.asarray(v_out_w, np.float32)
    v_out_b = np.asarray(v_out_b, np.float32)
    mlp_w1 = np.asarray(mlp_w1, np.float32)
    mlp_b1 = np.asarray(mlp_b1, np.float32)
    mlp_w2 = np.asarray(mlp_w2, np.float32)
    mlp_b2 = np.asarray(mlp_b2, np.float32)

    # V biases act as a constant shift of ctx (softmax weights sum to 1),
    # so fold them through the out-projections.
    h_out_eff = h_out_b + h_out_w @ h_in_b[2 * E:3 * E]
    v_out_eff = v_out_b + v_out_w @ v_in_b[2 * E:3 * E]

    # Fold the out-projections into the MLP's first matmul (exact algebra):
    # hid = relu(W1h @ (h_ctx @ Who.T + hob) + W1v @ (v_ctx @ Wvo.T + vob)
    #            + b1)
    #     = relu(h_ctx @ (W1h @ Who).T + v_ctx @ (W1v @ Wvo).T + b1_eff)
    w1h = mlp_w1[:, 0:E]
    w1v = mlp_w1[:, E:2 * E]
    w_m1_eff = np.concatenate([w1h @ h_out_w, w1v @ v_out_w], axis=1)
    b1_eff = mlp_b1 + w1h @ h_out_eff + w1v @ v_out_eff

    bias_flags = (
        bool(np.any(v_in_b[0:2 * E])), bool(np.any(h_in_b[0:E])),
        bool(np.any(h_in_b[E:2 * E])), bool(np.any(b1_eff)),
        bool(np.any(mlp_b2)),
    )
    nc = _get_program(bias_flags)

    biases = np.zeros((128, 32), np.float32)
    biases[:, 0:8] = _col(v_in_b[0:2 * E])
    biases[:, 8:16] = _col(h_in_b[0:2 * E])
    biases[:, 24:28] = _col(b1_eff)
    biases[:, 28:32] = _col(mlp_b2)

    shared = {
        "w_vin": np.ascontiguousarray(v_in_w.T).astype(NPBF),
        "w_hq": np.ascontiguousarray(h_in_w[0:E].T).astype(NPBF),
        "w_hkv": np.ascontiguousarray(h_in_w[E:3 * E].T).astype(NPBF),
        "w_m1": np.ascontiguousarray(w_m1_eff.T).astype(NPBF),
        "w_m2": np.ascontiguousarray(mlp_w2.T).astype(NPBF),
        "mask": _band_masks(),
        "biases": biases,
    }

    in_maps = []
    for c in range(NCORE):
        rows = x[RPC * c:RPC * (c + 1)]                      # (24, 192, 512)
        cols = x[:, RPC * c:RPC * (c + 1)].transpose(1, 0, 2)  # (24, 192, 512)
        m = dict(shared)
        m["xr_t"] = np.ascontiguousarray(rows.reshape(T, E).T).astype(NPBF)
        m["xc_t"] = np.ascontiguousarray(cols.reshape(T, E).T).astype(NPBF)
        in_maps.append(m)

    global _LAST_IN_MAPS
    _LAST_IN_MAPS = in_maps
    res = run_bass_kernel_spmd(nc, in_maps, core_ids=list(range(NCORE)))

    out = np.empty((S, S, E), np.float32)
    for c in range(NCORE):
        out[RPC * c:RPC * (c + 1)] = res.results[c]["out_t"].T.reshape(RPC, S, E)
    return out



# revision 19
# speedup vs baseline: 1.8798x; 1.0495x over previous
"""BiSPA (bidirectional sparse windowed attention + MLP) Trainium2 kernel.

Full inputs in, full outputs out. Internally shards across 8 NeuronCores:
core c owns output rows Ic = [24c, 24c+24) of the (192, 192, 512) grid.

Key observation: with B == S == 192 and window W == 32,
  - vertical attention for output row i is a complete 192-token sliding-window
    attention over x[i, :, :]                        (needs x rows  Ic)
  - horizontal attention for output row i is a complete 192-token
    sliding-window attention with Q from x[i, :, :] and K/V from x[:, i, :]
    (needs x columns Ic)
so each core needs x[Ic, :, :] and x[:, Ic, :] and NOTHING else -> zero
duplicated projection FLOPs, zero collectives, no halos.

Numerics: all matmul inputs bf16, fp32 PSUM accumulation, fp32 softmax exp
input, bf16 probs/ctx.  Measured end-to-end rel err vs fp32 reference ~0.6%.

Per strip (one row index, 192 tokens, processed identically for both
branches):
  QK^T feature-major (d on partitions)   -> scores S^T = K @ Q^T (keys on
  partitions, q on free), symmetric two-rectangle band cover (exact,
  disjoint):
     TA: keys[0:128)   x q[0:96)    (mask: pure band |k-q| <= W)
     TB: keys[64:192)  x q[96:192)  (mask: pure band)
  exp (ACT, scale=1/8, fp32->bf16) -> band-mask multiply (DVE) -> P
  V token-major with a ones-column per head (V') -> attn@V token-major,
  2 matmuls/head (the ones-column accumulates the softmax denominator Z
  into col 64 of each 65-col region; regions [h0q1|h1q1|h0q2|h1q2] on
  partitions [0:96) of one bank) -> ONE reciprocal + ONE free-broadcast
  tensor_tensor normalizes all four regions -> PE transpose to
  feature-major -> fused MLP.

The out-projections are folded into the MLP's first matmul on the host
(hid = relu(W1h' @ h_ctx + W1v' @ v_ctx + b1_eff) with W1h' = W1h @ h_out_w,
W1v' = W1v @ v_out_w) -- exact algebra, kills 14% of PE cycles.

Perf model (measured): every matmul pays ~100ns fixed LDWEIGHTS, so
PE time ~= sum(max(100ns, free_size x clock)) x throttle(~1.27); the
design minimizes matmul count (302/pair-group) and keeps ACT/DVE/Pool
off the critical path.
"""

import numpy as np
from contextlib import ExitStack

import concourse.bass as bass
import concourse.mybir as mybir
import concourse.tile as tile
from concourse import bacc
from concourse.bass_utils import run_bass_kernel_spmd
from concourse.masks import make_identity
from concourse.tile import add_dep_helper


def _chain(insts):
    """Order matmuls targeting one PSUM bank: a start=True zeroes (marks
    pending-zero) the WHOLE 2KB bank, so each bank must hold exactly one
    accumulation group and the group's matmuls must execute in program order.
    Tile won't order disjoint-region writes by itself."""
    for a, b in zip(insts, insts[1:]):
        add_dep_helper(b.ins, a.ins, sync=False, reason="psum-bank group order")

BF = mybir.dt.bfloat16
F32 = mybir.dt.float32
AF = mybir.ActivationFunctionType
MUL = mybir.AluOpType.mult
NPBF = mybir.dt.np(BF)

E = 512
H = 8
D = 64
W = 32
S = 192
NCORE = 8
RPC = 24          # rows (strips) per core
T = RPC * S       # tokens per core per branch = 4608

# ctx transpose path: "pe" (TensorE transpose) or "dma" (xbar DMA transpose)
CTX_TRANSPOSE = "pe"


def _band_masks():
    """Score mask, bf16 (128, 384): [TA 96 | TB 96] x 2 heads.

    Symmetric two-rectangle band cover (exact, no overlap):
    TA: rows p = key k in [0,128), cols q in [0,96).   valid = |k-q| <= W
    TB: rows p -> key k = 64+p in [64,192), cols q' -> q = 96+q' in
        [96,192).                                      valid = |k-q| <= W
    """
    m = np.zeros((128, 192), np.float32)
    k = np.arange(128)[:, None]
    q = np.arange(96)[None, :]
    m[:, 0:96] = (np.abs(k - q) <= W)
    kb = 64 + np.arange(128)[:, None]
    qb = 96 + np.arange(96)[None, :]
    m[:, 96:192] = (np.abs(kb - qb) <= W)
    return np.concatenate([m, m], axis=1).astype(NPBF)


def _build_program(bias_flags):
    """Build the SPMD Bass/Tile program (same program on all 8 cores)."""
    has_vqk_b, has_hq_b, has_hk_b, has_b1, has_b2 = bias_flags

    nc = bacc.Bacc("TRN2", target_bir_lowering=False, debug=False,
                   num_devices=NCORE, num_swdge_queues=4)

    xr_t = nc.dram_tensor("xr_t", [E, T], BF, kind="ExternalInput").ap()
    xc_t = nc.dram_tensor("xc_t", [E, T], BF, kind="ExternalInput").ap()
    w_vin = nc.dram_tensor("w_vin", [E, 3 * E], BF, kind="ExternalInput").ap()
    w_hq = nc.dram_tensor("w_hq", [E, E], BF, kind="ExternalInput").ap()
    w_hkv = nc.dram_tensor("w_hkv", [E, 2 * E], BF, kind="ExternalInput").ap()
    w_m1 = nc.dram_tensor("w_m1", [2 * E, E], BF, kind="ExternalInput").ap()
    w_m2 = nc.dram_tensor("w_m2", [E, E], BF, kind="ExternalInput").ap()
    mask_d = nc.dram_tensor("mask", [128, 384], BF, kind="ExternalInput").ap()
    bias_d = nc.dram_tensor("biases", [128, 32], F32, kind="ExternalInput").ap()
    out_t = nc.dram_tensor("out_t", [E, T], F32, kind="ExternalOutput").ap()

    with tile.TileContext(nc) as tc, ExitStack() as ctx:
        pw = ctx.enter_context(tc.tile_pool(name="pw", bufs=1))
        psA = ctx.enter_context(tc.tile_pool(name="psA", bufs=2, space="PSUM"))
        psS = ctx.enter_context(tc.tile_pool(name="psS", bufs=3, space="PSUM"))
        psC = ctx.enter_context(tc.tile_pool(name="psC", bufs=2, space="PSUM"))
        px = ctx.enter_context(tc.tile_pool(name="px", bufs=4))
        pqk = ctx.enter_context(tc.tile_pool(name="pqk", bufs=32))
        pv = ctx.enter_context(tc.tile_pool(name="pv", bufs=8))
        pp = ctx.enter_context(tc.tile_pool(name="pp", bufs=12))
        pctx = ctx.enter_context(tc.tile_pool(name="pctx", bufs=8))
        pzr = ctx.enter_context(tc.tile_pool(name="pzr", bufs=8))
        pct = ctx.enter_context(tc.tile_pool(name="pct", bufs=16))
        phid = ctx.enter_context(tc.tile_pool(name="phid", bufs=8))
        pout = ctx.enter_context(tc.tile_pool(name="pout", bufs=8))

        # ---- persistent constants ----
        # Issued on the sync (SP) engine so the weight loads run in
        # parallel with the gpsimd-issued x-tile loads of the first
        # pair-groups instead of serializing in front of them.
        def load_const(name, dram_ap, shape, dtype):
            t = pw.tile(shape, dtype, tag=name)
            nc.sync.dma_start(t[:], dram_ap)
            return t

        wv = [load_const(f"wv{k}", w_vin[128 * k:128 * (k + 1), :], [128, 3 * E], BF)
              for k in range(4)]
        whq = [load_const(f"whq{k}", w_hq[128 * k:128 * (k + 1), :], [128, E], BF)
               for k in range(4)]
        whkv = [load_const(f"whkv{k}", w_hkv[128 * k:128 * (k + 1), :], [128, 2 * E], BF)
                for k in range(4)]
        wm1 = [load_const(f"wm1{k}", w_m1[128 * k:128 * (k + 1), :], [128, E], BF)
               for k in range(8)]
        wm2 = [load_const(f"wm2{k}", w_m2[128 * k:128 * (k + 1), :], [128, E], BF)
               for k in range(4)]
        msk = load_const("msk", mask_d[:, :], [128, 384], BF)
        bia = load_const("bia", bias_d[:, :], [128, 32], F32)
        ident = pw.tile([128, 128], BF, tag="ident")
        make_identity(nc, ident)

        # bias column map (within `bia`):
        # 0-7 v_in_b[0:1024] ftiles; 8-11 h_in_b[0:512]; 12-15 h_in_b[512:1024]
        # 16-19 h_out_eff; 20-23 v_out_eff; 24-27 mlp_b1; 28-31 mlp_b2

        def evict384(ps, dst_pool, tag, dtype, func, has_bias, bias_base):
            """Evict a (128, 384) PSUM region (two 192-col ftile halves) to
            SBUF via ScalarE, optionally adding per-ftile per-partition bias."""
            dst = dst_pool.tile([128, 384], dtype, tag=tag)
            if has_bias:
                for half in range(2):
                    nc.scalar.activation(
                        dst[:, 192 * half:192 * half + 192],
                        ps[:, 192 * half:192 * half + 192],
                        func, bias=bia[:, bias_base + half:bias_base + half + 1])
            else:
                nc.scalar.activation(dst[:, 0:384], ps[:, 0:384], func)
            return dst

        import os as _os
        NPAIR = int(_os.environ.get("BISPA_NPAIRS", RPC // 2))
        for g in range(NPAIR):
            g0 = 2 * S * g
            # ---- stage X^T for the strip pair (384 tokens each) ----
            xr2 = []
            xc2 = []
            for k in range(4):
                t = px.tile([128, 2 * S], BF, tag=f"xr{k}")
                nc.gpsimd.dma_start(t[:], xr_t[128 * k:128 * (k + 1), g0:g0 + 2 * S])
                xr2.append(t)
                t = px.tile([128, 2 * S], BF, tag=f"xc{k}")
                nc.gpsimd.dma_start(t[:], xc_t[128 * k:128 * (k + 1), g0:g0 + 2 * S])
                xc2.append(t)

            # ---------- QK projections, feature-major, N=384 ----------
            # ftile j in 0..7: j<4 -> Q features, j>=4 -> K features
            qk = {}
            for br in ("h", "v"):
                qk[br] = []
                for j in range(8):
                    ps = psA.tile([128, 384], F32, tag="proj",
                                  padded_shape=[128, 512])
                    for k in range(4):
                        if br == "v":
                            lhsT = wv[k][:, 128 * j:128 * (j + 1)]
                            rhs = xr2[k][:]
                        elif j < 4:   # h Q
                            lhsT = whq[k][:, 128 * j:128 * (j + 1)]
                            rhs = xr2[k][:]
                        else:         # h K
                            lhsT = whkv[k][:, 128 * (j - 4):128 * (j - 3)]
                            rhs = xc2[k][:]
                        nc.tensor.matmul(ps[:], lhsT=lhsT, rhs=rhs,
                                         start=(k == 0), stop=(k == 3))
                    # bias columns: v ftiles 0-7 -> cols 0-7; h Q 0-3 -> 8-11;
                    # h K 0-3 -> 12-15
                    bcol = j if br == "v" else (8 + j)
                    dst = pqk.tile([128, 384], BF, tag="qk")
                    nc.scalar.activation(dst[:], ps[:], AF.Identity,
                                         bias=bia[:, bcol:bcol + 1])
                    qk[br].append(dst)

            ct = {"h": [], "v": []}   # ctx^T tiles (128, 384), per c-ftile
            for br in ("h", "v"):
                for p in range(4):
                    ct_t = pct.tile([128, 2 * S], BF, tag="ct", name=f"ct_{br}_{g}_{p}")
                    ct[br].append(ct_t)

            for a in range(2):        # strip within the pair
                s0 = S * a
                for br in ("h", "v"):
                    xin = xr2 if br == "v" else xc2
                    vcols = slice(1024, 1536) if br == "v" else slice(512, 1024)
                    vw = wv if br == "v" else whkv
                    # ------ V projection, token-major, with ones column ----
                    vps_a = psA.tile([128, 512], F32, tag="proj")
                    vps_b = psA.tile([128, 512], F32, tag="proj")
                    for k in range(4):
                        nc.tensor.matmul(vps_a[:], lhsT=xin[k][:, s0:s0 + 128],
                                         rhs=vw[k][:, vcols],
                                         start=(k == 0), stop=(k == 3))
                    for k in range(4):
                        nc.tensor.matmul(vps_b[:], lhsT=xin[k][:, s0 + 64:s0 + 192],
                                         rhs=vw[k][:, vcols],
                                         start=(k == 0), stop=(k == 3))
                    va = pv.tile([128, 8, 65], BF, tag="vp")   # keys [0:128)
                    vb = pv.tile([128, 8, 65], BF, tag="vp")   # keys [64:192)
                    nc.vector.tensor_copy(
                        va[:, :, 0:64],
                        vps_a[:].rearrange("p (h c) -> p h c", c=64))
                    nc.vector.tensor_copy(
                        vb[:, :, 0:64],
                        vps_b[:].rearrange("p (h c) -> p h c", c=64))
                    nc.vector.memset(va[:, :, 64:65], 1.0)
                    nc.vector.memset(vb[:, :, 64:65], 1.0)

                    # ------ attention, head-pair-wise ------
                    for p in range(4):
                        QT = qk[br][p][:, s0:s0 + S]
                        KT = qk[br][4 + p][:, s0:s0 + S]

                        # Scores per head in its OWN psum bank: matmuls with
                        # disjoint contraction row-groups (head0 at partitions
                        # 0:64, head1 at 64:128) run CONCURRENTLY on the PE
                        # and hard-fault if they write the same PSUM bank.
                        # Separate banks make the concurrency a ~2x PE win.
                        # Symmetric band cover: TA = keys[0:128) x q[0:96),
                        # TB = keys[64:192) x q[96:192), masks pure band.
                        sps = []
                        for h2 in range(2):
                            d0 = 64 * h2
                            sp = psS.tile([128, 512], F32, tag="sc")
                            nc.tensor.matmul(sp[:, 0:96],
                                             lhsT=KT[d0:d0 + 64, 0:128],
                                             rhs=QT[d0:d0 + 64, 0:96],
                                             start=True, stop=True)
                            nc.tensor.matmul(sp[:, 96:192],
                                             lhsT=KT[d0:d0 + 64, 64:192],
                                             rhs=QT[d0:d0 + 64, 96:192],
                                             start=True, stop=True)
                            sps.append(sp)
                        pb = pp.tile([128, 512], BF, tag="p")
                        for h2 in range(2):
                            nc.scalar.activation(pb[:, 192 * h2:192 * h2 + 192],
                                                 sps[h2][:, 0:192], AF.Exp,
                                                 scale=0.125)
                        pm = pp.tile([128, 512], BF, tag="p")
                        nc.vector.tensor_tensor(pm[:, 0:384], pb[:, 0:384],
                                                msk[:, 0:384], op=MUL)

                        # attn@V: 2 matmuls per head. q[0:96) from keys
                        # [0:128) (va), q[96:192) from keys [64:192) (vb);
                        # all write partitions [0:96) of one bank in 65-col
                        # regions [h0q1 | h1q1 | h0q2 | h1q2] (col 64 of
                        # each = Z from the ones column). One accumulation
                        # "group" per bank, ordered by _chain; the group
                        # checker cannot express multi-region banks, so
                        # skip it.
                        cp = psC.tile([128, 512], F32, tag="cx")
                        mms = []
                        for h2 in range(2):
                            h = 2 * p + h2
                            ta = 192 * h2
                            tb = 192 * h2 + 96
                            mms.append(nc.tensor.matmul(
                                cp[0:96, 65 * h2:65 * h2 + 65],
                                lhsT=pm[:, ta:ta + 96],
                                rhs=va[:, h:h + 1, :], start=(h2 == 0),
                                stop=False, skip_group_check=True))
                            mms.append(nc.tensor.matmul(
                                cp[0:96, 130 + 65 * h2:195 + 65 * h2],
                                lhsT=pm[:, tb:tb + 96],
                                rhs=vb[:, h:h + 1, :],
                                start=False, stop=(h2 == 1),
                                skip_group_check=True))
                        _chain(mms)

                        # normalize by 1/Z, one reciprocal + one broadcast
                        # multiply: ctxn = [h0q1 | h1q1 | h0q2 | h1q2] on
                        # partitions [0:96), transpose-ready.
                        zr = pzr.tile([128, 4, 1], F32, tag="zr")
                        cp4 = cp[0:96, 0:260].rearrange("p (x c) -> p x c",
                                                        c=65)
                        ctxn = pctx.tile([128, 256], BF, tag="ctxn")
                        reads = [
                            nc.vector.reciprocal(zr[0:96, :, :],
                                                 cp4[:, :, 64:65]),
                            nc.vector.tensor_tensor(
                                ctxn[0:96, 0:256].rearrange(
                                    "p (a b) -> p a b", b=64),
                                cp4[:, :, 0:64],
                                zr[0:96, :, :].broadcast_to([96, 4, 64]),
                                op=MUL),
                        ]
                        # cp reads must wait for the accumulation group to
                        # close (same-bank PE-write + DVE-read is a HW fault)
                        for r in reads:
                            add_dep_helper(r.ins, mms[-1].ins, sync=True,
                                           reason="psum read after group close")

                        # transpose q1/q2 blocks into ct (feature-major)
                        ct_p = ct[br][p]
                        ctp = psC.tile([128, S], BF, tag="cxT", bufs=1)
                        nc.tensor.transpose(ctp[:, 0:96], ctxn[0:96, 0:128],
                                            ident[0:96, 0:96])
                        nc.tensor.transpose(ctp[:, 96:192],
                                            ctxn[0:96, 128:256],
                                            ident[0:96, 0:96])
                        # DVE (not ACT) for the slack-rich copy into ct:
                        # keeps the scalar engine free for exp + evicts.
                        nc.vector.tensor_copy(ct_p[:, s0:s0 + S], ctp[:])

            # ---------- fused out-proj + MLP1, N=384 ----------
            # hid = relu(W1h' @ h_ctx + W1v' @ v_ctx + b1_eff), where
            # W1h' = W1[:, 0:E] @ h_out_w and W1v' = W1[:, E:2E] @ v_out_w
            # are folded on the host. wm1 rows 0:512 act on h ctx,
            # rows 512:1024 on v ctx.
            hid = []
            for j in range(4):
                ps = psA.tile([128, 384], F32, tag="proj",
                              padded_shape=[128, 512])
                for k in range(8):
                    rhs = ct["h"][k] if k < 4 else ct["v"][k - 4]
                    nc.tensor.matmul(ps[:],
                                     lhsT=wm1[k][:, 128 * j:128 * (j + 1)],
                                     rhs=rhs[:],
                                     start=(k == 0), stop=(k == 7))
                dst = phid.tile([128, 384], BF, tag="hid")
                nc.scalar.activation(dst[:], ps[:], AF.Relu,
                                     bias=bia[:, 24 + j:24 + j + 1])
                hid.append(dst)
            for j in range(4):
                ps = psA.tile([128, 384], F32, tag="proj",
                              padded_shape=[128, 512])
                for k in range(4):
                    nc.tensor.matmul(ps[:],
                                     lhsT=wm2[k][:, 128 * j:128 * (j + 1)],
                                     rhs=hid[k][:],
                                     start=(k == 0), stop=(k == 3))
                osb = pout.tile([128, 384], F32, tag="o")
                nc.scalar.activation(osb[:], ps[:], AF.Identity,
                                     bias=bia[:, 28 + j:28 + j + 1])
                nc.sync.dma_start(out_t[128 * j:128 * (j + 1), g0:g0 + 2 * S],
                                  osb[:])
    nc.finalize()
    return nc


_CACHE = {}


def _get_program(bias_flags):
    key = tuple(bias_flags)
    if key not in _CACHE:
        _CACHE[key] = _build_program(key)
    return _CACHE[key]


def _col(b):
    """bias vector (128*n,) -> (128, n) column-pack, fortran-ish layout."""
    return np.ascontiguousarray(b.reshape(-1, 128).T.astype(np.float32))


def kernel(hidden_states, h_in_w, h_in_b, h_out_w, h_out_b,
           v_in_w, v_in_b, v_out_w, v_out_b,
           mlp_w1, mlp_b1, mlp_w2, mlp_b2):
    x = np.asarray(hidden_states, dtype=np.float32)
    h_in_w = np.asarray(h_in_w, np.float32)
    h_in_b = np.asarray(h_in_b, np.float32)
    h_out_w = np.asarray(h_out_w, np.float32)
    h_out_b = np.asarray(h_out_b, np.float32)
    v_in_w = np.asarray(v_in_w, np.float32)
    v_in_b = np.asarray(v_in_b, np.float32)
    v_out_w = np.asarray(v_out_w, np.float32)
    v_out_b = np.asarray(v_out_b, np.float32)
    mlp_w1 = np.asarray(mlp_w1, np.float32)
    mlp_b1 = np.asarray(mlp_b1, np.float32)
    mlp_w2 = np.asarray(mlp_w2, np.float32)
    mlp_b2 = np.asarray(mlp_b2, np.float32)

    # V biases act as a constant shift of ctx (softmax weights sum to 1),
    # so fold them through the out-projections.
    h_out_eff = h_out_b + h_out_w @ h_in_b[2 * E:3 * E]
    v_out_eff = v_out_b + v_out_w @ v_in_b[2 * E:3 * E]

    # Fold the out-projections into the MLP's first matmul (exact algebra):
    # hid = relu(W1h @ (h_ctx @ Who.T + hob) + W1v @ (v_ctx @ Wvo.T + vob)
    #            + b1)
    #     = relu(h_ctx @ (W1h @ Who).T + v_ctx @ (W1v @ Wvo).T + b1_eff)
    w1h = mlp_w1[:, 0:E]
    w1v = mlp_w1[:, E:2 * E]
    w_m1_eff = np.concatenate([w1h @ h_out_w, w1v @ v_out_w], axis=1)
    b1_eff = mlp_b1 + w1h @ h_out_eff + w1v @ v_out_eff

    bias_flags = (
        bool(np.any(v_in_b[0:2 * E])), bool(np.any(h_in_b[0:E])),
        bool(np.any(h_in_b[E:2 * E])), bool(np.any(b1_eff)),
        bool(np.any(mlp_b2)),
    )
    nc = _get_program(bias_flags)

    biases = np.zeros((128, 32), np.float32)
    biases[:, 0:8] = _col(v_in_b[0:2 * E])
    biases[:, 8:16] = _col(h_in_b[0:2 * E])
    biases[:, 24:28] = _col(b1_eff)
    biases[:, 28:32] = _col(mlp_b2)

    shared = {
        "w_vin": np.ascontiguousarray(v_in_w.T).astype(NPBF),
        "w_hq": np.ascontiguousarray(h_in_w[0:E].T).astype(NPBF),
        "w_hkv": np.ascontiguousarray(h_in_w[E:3 * E].T).astype(NPBF),
        "w_m1": np.ascontiguousarray(w_m1_eff.T).astype(NPBF),
        "w_m2": np.ascontiguousarray(mlp_w2.T).astype(NPBF),
        "mask": _band_masks(),
        "biases": biases,
    }

    in_maps = []
    for c in range(NCORE):
        rows = x[RPC * c:RPC * (c + 1)]                      # (24, 192, 512)
        cols = x[:, RPC * c:RPC * (c + 1)].transpose(1, 0, 2)  # (24, 192, 512)
        m = dict(shared)
        m["xr_t"] = np.ascontiguousarray(rows.reshape(T, E).T).astype(NPBF)
        m["xc_t"] = np.ascontiguousarray(cols.reshape(T, E).T).astype(NPBF)
        in_maps.append(m)

    global _LAST_IN_MAPS
    _LAST_IN_MAPS = in_maps
    res = run_bass_kernel_spmd(nc, in_maps, core_ids=list(range(NCORE)))

    out = np.empty((S, S, E), np.float32)
    for c in range(NCORE):
        out[RPC * c:RPC * (c + 1)] = res.results[c]["out_t"].T.reshape(RPC, S, E)
    return out

